# revision 11
# baseline (speedup 1.0000x reference)
"""Trainium2 Bass kernel for causal multi-head attention with full-dim rotary.

Computes, for inputs q,k,v [B=2, L=2048, D=1024] and weights Wq/Wk/Wv/Wo [D,D]:
    Q = rope(q @ Wq.T + bq); K = rope(k @ Wk.T + bk); V = v @ Wv.T + bv
    out = softmax_causal(Qh Kh^T / sqrt(dh)) Vh  (H=16 heads, dh=64)
    y = out @ Wo.T + bo

Sharding: 8 cores = (batch b in {0,1}) x (block of 4 heads). Each core computes
its 4 heads' Q/K/V projections (256 output features), runs causal flash
attention in S^T orientation (softmax denominator fused as a ones-column of V),
and emits a partial output projection y_partial [L, D]. The host sums the 4
partials per batch and adds the bias correction row (bv @ Wo.T + bo).

Numerics: bf16 matmul operands with fp32 PSUM accumulation throughout; exp on
the scalar engine in fp32 (reads PSUM scores directly, scale=1/8 fused).
"""

import sys
import functools
import numpy as np

try:
    import concourse.bass as bass
except ImportError:  # fresh grading dir: concourse lives in the container image
    sys.path.insert(0, "/opt/trn_rl_repo")
    import concourse.bass as bass

import ml_dtypes
import concourse.mybir as mybir
import concourse.tile as tile
from concourse import bacc
from concourse.bass_utils import run_bass_kernel_spmd

BF16 = mybir.dt.bfloat16
F32 = mybir.dt.float32
AF = mybir.ActivationFunctionType

B, D, H, DH = 2, 1024, 16, 64
P = 128
KT = D // P            # 8 contraction tiles for the projections
HPC = 4                # heads per core
NPR = 2                # head-pairs per core
N_CORES = 8
MAXPOS = 10000.0
MCH = 512              # m-chunk (rows) for the projection phase


def _split512(lo, hi):
    """Split [lo, hi) at global multiples of 512."""
    out = []
    s = lo
    while s < hi:
        e = min(hi, (s // 512 + 1) * 512)
        out.append((s, e))
        s = e
    return out


def build_program(L, qk_bias=False):
    NT = L // P                       # key tiles
    MC = min(MCH, L)
    NMCH = L // MC
    passes = [(0, min(L, 1024))]
    if L > 1024:
        passes.append((1024, L))

    nc = bacc.Bacc(None)
    xq = nc.declare_dram_parameter("xq", [D, L], BF16, isOutput=False)
    xk = nc.declare_dram_parameter("xk", [D, L], BF16, isOutput=False)
    xv = nc.declare_dram_parameter("xv", [D, L], BF16, isOutput=False)
    wq = nc.declare_dram_parameter("wq", [D, 2 * P], BF16, isOutput=False)
    wk = nc.declare_dram_parameter("wk", [D, 2 * P], BF16, isOutput=False)
    wv = nc.declare_dram_parameter("wv", [D, 2 * P], BF16, isOutput=False)
    wo = nc.declare_dram_parameter("wo", [2 * P, D], BF16, isOutput=False)
    ctab = nc.declare_dram_parameter("ctab", [NPR, P, L], BF16, isOutput=False)
    stab = nc.declare_dram_parameter("stab", [NPR, P, L], BF16, isOutput=False)
    if qk_bias:
        rqt = nc.declare_dram_parameter("rqt", [NPR, P, L], BF16, isOutput=False)
        rkt = nc.declare_dram_parameter("rkt", [NPR, P, L], BF16, isOutput=False)
    tri = nc.declare_dram_parameter("tri", [P, P], BF16, isOutput=False)
    y = nc.declare_dram_parameter("y", [L, D], F32, isOutput=True)

    with tile.TileContext(nc) as tc:
        from contextlib import ExitStack

        with ExitStack() as ctx:
            consts = ctx.enter_context(tc.tile_pool(name="consts", bufs=1))
            qk_sb = ctx.enter_context(tc.tile_pool(name="qk_sb", bufs=1))
            xin = ctx.enter_context(tc.tile_pool(name="xin", bufs=2))
            tmp = ctx.enter_context(tc.tile_pool(name="tmp", bufs=3))
            pts = ctx.enter_context(tc.tile_pool(name="pts", bufs=3))
            small = ctx.enter_context(tc.tile_pool(name="small", bufs=3))

            # ---- constants ----
            wq_sb = consts.tile([P, KT, 2 * P], BF16, tag="wq")
            wk_sb = consts.tile([P, KT, 2 * P], BF16, tag="wk")
            wv_sb = consts.tile([P, KT, 2 * P], BF16, tag="wv")
            wo_sb = consts.tile([P, NPR, D], BF16, tag="wo")
            c_sb = consts.tile([P, NPR, L], BF16, tag="ctab")
            s_sb = consts.tile([P, NPR, L], BF16, tag="stab")
            if qk_bias:
                rq_sb = consts.tile([P, NPR, L], BF16, tag="rqt")
                rk_sb = consts.tile([P, NPR, L], BF16, tag="rkt")
                nc.sync.dma_start(rq_sb[:], rqt[:].rearrange("pr p l -> p pr l"))
                nc.sync.dma_start(rk_sb[:], rkt[:].rearrange("pr p l -> p pr l"))
            else:
                rq_sb = rk_sb = None
            tri_sb = consts.tile([P, P], BF16, tag="tri")
            ones64 = consts.tile([1, DH], BF16, tag="ones64")
            nc.vector.memset(ones64[:], 1.0)
            nc.sync.dma_start(wq_sb[:], wq[:].rearrange("(kt p) n -> p kt n", p=P))
            nc.sync.dma_start(wk_sb[:], wk[:].rearrange("(kt p) n -> p kt n", p=P))
            nc.sync.dma_start(wv_sb[:], wv[:].rearrange("(kt p) n -> p kt n", p=P))
            nc.sync.dma_start(wo_sb[:], wo[:].rearrange("(pr p) n -> p pr n", p=P))
            nc.sync.dma_start(c_sb[:], ctab[:].rearrange("pr p l -> p pr l"))
            nc.sync.dma_start(s_sb[:], stab[:].rearrange("pr p l -> p pr l"))
            nc.sync.dma_start(tri_sb[:], tri[:])

            # persistent activations
            QT = [qk_sb.tile([P, L], BF16, tag=f"QT{pr}", name=f"QT{pr}") for pr in range(NPR)]
            KTt = [qk_sb.tile([P, L], BF16, tag=f"KT{pr}", name=f"KT{pr}") for pr in range(NPR)]
            # V' per (j-tile, head): 64 cols of V + a ones column
            Vp = qk_sb.tile([P, NT, HPC, DH + 1], BF16, tag="Vp")
            OT = [qk_sb.tile([P, L], BF16, tag=f"OT{pr}", name=f"OT{pr}") for pr in range(NPR)]
            nc.vector.memset(Vp[:, :, :, DH : DH + 1], 1.0)

            # a<->b half swap within each 32-partition quadrant
            SWAP = [(i + 16) % 32 for i in range(32)]

            xq_r = xq[:].rearrange("(kt p) l -> p kt l", p=P)
            xk_r = xk[:].rearrange("(kt p) l -> p kt l", p=P)
            xv_r = xv[:].rearrange("(kt p) l -> p kt l", p=P)

            # ================= Phase 1: projections + rope =================
            with tc.tile_pool(name="pp", bufs=4, space="PSUM") as pp:
                for m in range(NMCH):
                    ms = m * MC
                    xq_t = xin.tile([P, KT, MC], BF16, tag="xq")
                    xk_t = xin.tile([P, KT, MC], BF16, tag="xk")
                    xv_t = xin.tile([P, KT, MC], BF16, tag="xv")
                    nc.sync.dma_start(xq_t[:], xq_r[:, :, ms : ms + MC])
                    nc.sync.dma_start(xk_t[:], xk_r[:, :, ms : ms + MC])
                    nc.sync.dma_start(xv_t[:], xv_r[:, :, ms : ms + MC])

                    for pr in range(NPR):
                        for name, w_sb, x_t, r_sb, dst in (
                            ("q", wq_sb, xq_t, rq_sb, QT),
                            ("k", wk_sb, xk_t, rk_sb, KTt),
                        ):
                            ps = pp.tile([P, MC], F32, tag="ps_qk")
                            for kt in range(KT):
                                nc.tensor.matmul(
                                    ps[:],
                                    lhsT=w_sb[:, kt, pr * P : pr * P + P],
                                    rhs=x_t[:, kt, :],
                                    start=(kt == 0),
                                    stop=(kt == KT - 1),
                                )
                            # rope: t1 = (ps + b) * cos ; t2 = (ps + b) * sin*
                            t1 = tmp.tile([P, MC], BF16, tag="t1")
                            t2 = tmp.tile([P, MC], BF16, tag="t2")
                            t2s = tmp.tile([P, MC], BF16, tag="t2s")
                            nc.vector.tensor_mul(
                                t1[:], ps[:], c_sb[:, pr, ms : ms + MC]
                            )
                            nc.vector.tensor_mul(
                                t2[:], ps[:], s_sb[:, pr, ms : ms + MC]
                            )
                            nc.vector.stream_shuffle(t2s[:], t2[:], SWAP)
                            if qk_bias:
                                t3 = tmp.tile([P, MC], BF16, tag="t3")
                                nc.vector.tensor_add(t3[:], t1[:], t2s[:])
                                nc.vector.tensor_add(
                                    dst[pr][:, ms : ms + MC], t3[:],
                                    r_sb[:, pr, ms : ms + MC],
                                )
                            else:
                                nc.vector.tensor_add(
                                    dst[pr][:, ms : ms + MC], t1[:], t2s[:]
                                )

                    # V projection: row-major [m, n]
                    for msub in range(MC // P):
                        ps_v = pp.tile([P, 2 * P], F32, tag="ps_v")
                        for kt in range(KT):
                            nc.tensor.matmul(
                                ps_v[:],
                                lhsT=xv_t[:, kt, msub * P : msub * P + P],
                                rhs=wv_sb[:, kt, :],
                                start=(kt == 0),
                                stop=(kt == KT - 1),
                            )
                        jt = m * (MC // P) + msub
                        nc.scalar.activation(
                            Vp[:, jt, :, 0:DH], ps_v[:].rearrange("p (h d) -> p h d", d=DH),
                            AF.Copy,
                        )

            # ================= Phase 2: attention =================
            with (
                tc.tile_pool(name="ps_s", bufs=1, space="PSUM") as psum_s,
                tc.tile_pool(name="ps_o", bufs=1, space="PSUM") as psum_o,
            ):
                for pr in range(NPR):
                    for (p0, p1) in passes:
                        nch = (p1 - p0 + 511) // 512
                        po = [
                            [
                                psum_o.tile([P, min(512, p1 - p0 - 512 * lc)],
                                            F32, tag=f"o{h2}_{lc}", name=f"o{h2}_{lc}")
                                for lc in range(nch)
                            ]
                            for h2 in range(2)
                        ]
                        t_max = (p1 + P - 1) // P
                        for t in range(t_max):
                            rs = max(P * t, p0)
                            w = p1 - rs
                            i0 = (rs // 512) * 512  # bank-aligned tile anchor
                            for h2 in range(2):
                                hr = DH * h2
                                ps = psum_s.tile([P, p1 - i0], F32, tag=f"s{h2}")
                                for (s0, s1) in _split512(rs, p1):
                                    nc.tensor.matmul(
                                        ps[:, s0 - i0 : s1 - i0],
                                        lhsT=KTt[pr][hr : hr + DH, P * t : P * t + P],
                                        rhs=QT[pr][hr : hr + DH, s0:s1],
                                        start=True, stop=True,
                                    )
                                pt = pts.tile([P, w], BF16, tag=f"pt{h2}")
                                nc.scalar.activation(
                                    pt[:], ps[:, rs - i0 :], AF.Exp, scale=0.125
                                )
                                if rs == P * t:
                                    nc.vector.tensor_mul(
                                        pt[:, 0:P], pt[:, 0:P], tri_sb[:]
                                    )
                                for (s0, s1) in _split512(rs, p1):
                                    lc = (s0 - p0) // 512
                                    co = (s0 - p0) % 512
                                    tstop = min(t_max, (p0 + 512 * lc + 512) // P) - 1
                                    nc.tensor.matmul(
                                        po[h2][lc][0 : DH + 1, co : co + (s1 - s0)],
                                        lhsT=Vp[:, t, 2 * pr + h2, :],
                                        rhs=pt[:, s0 - rs : s1 - rs],
                                        start=(t == 0),
                                        stop=(t == tstop),
                                        skip_group_check=True,
                                    )
                        # normalize: OT = po[0:64] / po[64]
                        for h2 in range(2):
                            for lc in range(nch):
                                cw = min(512, p1 - p0 - 512 * lc)
                                cs = p0 + 512 * lc
                                r_t = small.tile([1, cw], BF16, tag="r")
                                with nc.allow_low_precision("softmax denom in bf16"):
                                    nc.vector.reciprocal(
                                        r_t[:], po[h2][lc][DH : DH + 1, :]
                                    )
                                # broadcast 1/l across partitions 64:128 via
                                # a K=1 ones matmul into the same PSUM bank
                                nc.tensor.matmul(
                                    po[h2][lc][DH : DH + DH, :],
                                    lhsT=ones64[0:1, :],
                                    rhs=r_t[:],
                                    start=True, stop=True,
                                )
                                rb_t = small.tile([DH, cw], F32, tag="rb")
                                nc.vector.tensor_copy(
                                    rb_t[:], po[h2][lc][DH : DH + DH, :]
                                )
                                nc.vector.tensor_mul(
                                    OT[pr][DH * h2 : DH * h2 + DH, cs : cs + cw],
                                    po[h2][lc][0:DH, :],
                                    rb_t[:],
                                )

            # ================= Phase 3: output projection =================
            with tc.tile_pool(name="ps_y", bufs=4, space="PSUM") as psum_y:
                for it in range(L // P):
                    for nc2 in range(D // 512):
                        ps_y = psum_y.tile([P, 512], F32, tag="y")
                        for pr in range(NPR):
                            nc.tensor.matmul(
                                ps_y[:],
                                lhsT=OT[pr][:, it * P : it * P + P],
                                rhs=wo_sb[:, pr, nc2 * 512 : nc2 * 512 + 512],
                                start=(pr == 0),
                                stop=(pr == NPR - 1),
                            )
                        y_t = tmp.tile([P, 512], F32, tag="ysb")
                        nc.vector.tensor_copy(y_t[:], ps_y[:])
                        nc.sync.dma_start(
                            y[it * P : it * P + P, nc2 * 512 : nc2 * 512 + 512],
                            y_t[:],
                        )
    nc.compile()
    return nc


@functools.lru_cache(maxsize=2)
def _get_program(L, qk_bias=False):
    return build_program(L, qk_bias)


def _rope_perm(hloc):
    """Column order (within this core's 256 outputs) for head-local index hloc.

    Row r (0..63) of head h: quadrant q = r//32, i = r%32.
    i < 16  -> even dim of freq 16q+i       (a half)
    i >= 16 -> odd dim  of freq 16q+(i-16)  (b half)
    Returns indices into the head's 64 original dims.
    """
    idx = np.zeros(64, dtype=np.int64)
    for r in range(64):
        q, i = divmod(r, 32)
        if i < 16:
            idx[r] = 2 * (16 * q + i)
        else:
            idx[r] = 2 * (16 * q + (i - 16)) + 1
    return idx


def _prep_core_inputs(c, L, q, k, v, Wq, bq, Wk, bk, Wv, bv, Wo, bo):
    b = c // (N_CORES // B)
    hb = HPC * (c % (N_CORES // B))  # first global head on this core
    bf = ml_dtypes.bfloat16

    xq = np.ascontiguousarray(q[b].T).astype(bf)
    xk = np.ascontiguousarray(k[b].T).astype(bf)
    xv = np.ascontiguousarray(v[b].T).astype(bf)

    # permuted row order of Wq/Wk for this core's 4 heads
    rows = np.concatenate(
        [64 * (hb + hl) + _rope_perm(hl) for hl in range(HPC)]
    )
    wq_t = np.ascontiguousarray(Wq[rows, :].T).astype(bf)        # [D, 256]
    wk_t = np.ascontiguousarray(Wk[rows, :].T).astype(bf)
    bq_p = bq[rows].astype(np.float64).reshape(NPR, P)
    bk_p = bk[rows].astype(np.float64).reshape(NPR, P)
    vrows = np.arange(64 * hb, 64 * (hb + HPC))
    wv_t = np.ascontiguousarray(Wv[vrows, :].T).astype(bf)       # [D, 256]
    wo_t = np.ascontiguousarray(Wo[:, vrows].T).astype(bf)       # [256, D]

    # rope tables in permuted row order; sin negated on b halves
    pos = np.arange(L, dtype=np.float64)
    ct = np.zeros((NPR, P, L), dtype=np.float64)
    st = np.zeros((NPR, P, L), dtype=np.float64)
    for pr in range(NPR):
        for h2 in range(2):
            hg = hb + 2 * pr + h2
            for r in range(64):
                qd, i = divmod(r, 32)
                f = 32 * hg + 16 * qd + (i % 16)
                theta = MAXPOS ** (-f / (D // 2))
                ang = pos * theta
                row = DH * h2 + r
                ct[pr, row] = np.cos(ang)
                st[pr, row] = np.sin(ang) if i < 16 else -np.sin(ang)
    ctab = ct.astype(np.float32).astype(bf)
    stab = st.astype(np.float32).astype(bf)

    jj = np.arange(P)
    tri = (jj[None, :] >= jj[:, None]).astype(np.float32).astype(bf)  # [j, i] keep i>=j

    im = {
        "xq": xq, "xk": xk, "xv": xv,
        "wq": wq_t, "wk": wk_t, "wv": wv_t, "wo": wo_t,
        "ctab": ctab, "stab": stab, "tri": tri,
    }
    if np.abs(bq).max() > 0 or np.abs(bk).max() > 0:
        def swap16(a):
            a4 = a.reshape(NPR, P // 32, 2, 16, L)
            return a4[:, :, ::-1, :, :].reshape(NPR, P, L)

        rqt = bq_p[:, :, None] * ct + swap16(bq_p[:, :, None] * st)
        rkt = bk_p[:, :, None] * ct + swap16(bk_p[:, :, None] * st)
        im["rqt"] = rqt.astype(np.float32).astype(bf)
        im["rkt"] = rkt.astype(np.float32).astype(bf)
    return im


def kernel(q, k, v, Wq, bq, Wk, bk, Wv, bv, Wo, bo):
    q, k, v = (np.asarray(a, dtype=np.float32) for a in (q, k, v))
    Wq, bq, Wk, bk, Wv, bv, Wo, bo = (
        np.asarray(a, dtype=np.float32) for a in (Wq, bq, Wk, bk, Wv, bv, Wo, bo)
    )
    Bq, L, Dq = q.shape
    assert (Bq, Dq) == (B, D)

    qk_bias = bool(np.abs(bq).max() > 0 or np.abs(bk).max() > 0)
    nc = _get_program(L, qk_bias)
    in_maps = [
        _prep_core_inputs(c, L, q, k, v, Wq, bq, Wk, bk, Wv, bv, Wo, bo)
        for c in range(N_CORES)
    ]
    res = run_bass_kernel_spmd(nc, in_maps, core_ids=list(range(N_CORES)))

    corr = (bv @ Wo.T + bo).astype(np.float32)  # folded-out V/O biases
    y = np.zeros((B, L, D), dtype=np.float32)
    cpb = N_CORES // B
    for c in range(N_CORES):
        y[c // cpb] += np.asarray(res.results[c]["y"], dtype=np.float32)
    y += corr[None, None, :]
    return y
